# revision 14
# baseline (speedup 1.0000x reference)
"""AttentionMM kernel for Trainium2 (Bass/Tile), data-parallel over 8 NeuronCores.

Math (per batch b, with x1,x2: (T,E)):
    S = x1 @ x2^T  is never materialized:
        psA = x1^T @ [x2|1] = [G2 | t1] ;  psB = x2^T @ [x1|1] = [G | t2]
        c1 = (1/T) G2^T t2 ;  c2 = (1/T) G t1
    et1 = c1 @ U1 + x1 @ W1 + b1 ;  et2 = c2 @ U2 + x2 @ W2 + b2
    o1 = softmax(et1) @ x1 ;  o2 = softmax(et2) @ x2 ;  out = [o1 | o2]

v12 (v10 was 47.9us, v2 54.4us, f32r baseline 83-91us):
  - x/U/W fp16, EX bf16 with the constant-shift softmax.
  - x@W moved to the PE: the host ships a SECOND, E-major (transposed)
    copy of x in fp8e4m3 (+2.1MB DMA into ~17us of DMA slack). Each
    U-phase k-tile then accumulates U_k^T @ c (fp16) and xT8_k^T @ W8
    (fp8, start=False) into the same PSUM column — deleting the entire
    ~23us DVE mult+reduce chain that was the critical path. Host sim:
    rel err 1.55e-2 (xW term quantized fp8; harness gate 2e-2). The
    fp8 bits are produced on the host, so hardware matches the sim.
  - Column-form readout (1-col moving operand), Z via activation
    accum_out; batched finalize: DVE 32x32 block transposes + ACT
    1/Z scale + a single 4KB output store.
  - U-phase: pair (b0,b1), then b2, b3 solo; xT copies are interleaved
    into the DMA stream right after each batch's fp16 tiles.
  - 60 wide warmup matmuls release the PE HAM throttle; TileContext
    exit skips the (unneeded) SWDGE sem reset.
"""

import numpy as np

import concourse.bass as bass
import concourse.mybir as mybir
import concourse.tile as tile
from concourse.bass_utils import run_bass_kernel_spmd

B, T, E = 32, 2048, 128
NCORES = 8
BPC = B // NCORES
KT = T // 128                # token t = p*16 + k
CW = E + 2
F32 = mybir.dt.float32
F16 = mybir.dt.float16
BF16 = mybir.dt.bfloat16
AF = mybir.ActivationFunctionType
ALU = mybir.AluOpType
ET_SHIFT = -40.0
INV_T = 1.0 / T
N_WARM = 60

# dtype of the E-major x copy used for the x@W PE matmuls.
# fp8e4m3: +2.1MB DMA, sim rel err 1.55e-2 (gate 2e-2).
# bf16 fallback: +4.3MB DMA, sim rel err ~3.1e-3.
XT_FP8 = True
XTDT = mybir.dt.float8e4 if XT_FP8 else BF16


def _xt_np_dtype():
    if XT_FP8:
        import ml_dtypes
        return ml_dtypes.float8_e4m3fn
    import ml_dtypes
    return ml_dtypes.bfloat16


def _patch_sem_clear():
    """The installed walrus cannot encode EVENT_SEMAPHORE_RANGE_CLEAR (raw
    ISA, "ISA wrong length"), which TileContext's exit path emits via
    gpsimd.sem_clear. Skip the clear entirely (keep the bookkeeping): the
    runtime re-initializes semaphore state per NEFF execution, and the
    re-execution check in test.py guards this assumption."""
    if getattr(bass.Bass, "_semclear_patched", False):
        return

    def patched(self, sems):
        if not sems:
            return
        sem_nums = [s.num if hasattr(s, "num") else s for s in sems]
        self._state.prepend_free_semaphores(sem_nums)
        for poison_set in self._tile_sem_poison_stack:
            poison_set.update(sem_nums)

    bass.Bass.clear_and_free_semaphores = patched
    bass.Bass._semclear_patched = True


def _legalize_sync_waits(nc):
    """The installed walrus encodes at most one sync-wait per instruction
    ("Too many sync wait commands"). Move excess waits onto engine NoOps
    inserted immediately before the instruction — same engine, same
    program position, so semantics are unchanged."""
    import bass_rust

    fn = nc.m.functions[0]
    n_nops = 0
    for blk in fn.blocks:
        insts = blk.instructions
        out = []
        dirty = False
        for inst in insts:
            si = inst.sync_info
            if si is not None and len(si.on_wait) > 1:
                waits = list(si.on_wait)
                for w in waits[:-1]:
                    nop = mybir.InstNoOp(
                        name=f"waitnop-{n_nops}", engine=inst.engine
                    )
                    nop.sync_info = bass_rust.SyncInfo(
                        on_wait=[w], on_update=[]
                    )
                    out.append(nop)
                    n_nops += 1
                inst.sync_info = bass_rust.SyncInfo(
                    on_wait=[waits[-1]], on_update=list(si.on_update)
                )
                dirty = True
            out.append(inst)
        if dirty:
            blk.instructions = out
    return n_nops


def _build():
    _patch_sem_clear()
    nc = bass.Bass(
        "TRN2", target_bir_lowering=False, debug=False, num_devices=NCORES
    )

    x1d = nc.dram_tensor("x1c", (BPC, T, CW), F16, kind="ExternalInput").ap()
    x2d = nc.dram_tensor("x2c", (BPC, T, CW), F16, kind="ExternalInput").ap()
    xt1d = nc.dram_tensor("x1t8", (BPC, E, T), XTDT, kind="ExternalInput").ap()
    xt2d = nc.dram_tensor("x2t8", (BPC, E, T), XTDT, kind="ExternalInput").ap()
    u1d = nc.dram_tensor("u1", (E, T), F16, kind="ExternalInput").ap()
    u2d = nc.dram_tensor("u2", (E, T), F16, kind="ExternalInput").ap()
    w8d = nc.dram_tensor("w8", (E, 2), XTDT, kind="ExternalInput").ap()
    bsd = nc.dram_tensor("bs", (128, 2 * KT), F32, kind="ExternalInput").ap()
    outd = nc.dram_tensor("out", (2 * BPC, E), F32, kind="ExternalOutput").ap()

    with tile.TileContext(nc) as tc:
        with (
            tc.tile_pool(name="const", bufs=1) as cpool,
            tc.tile_pool(name="xpool", bufs=1) as xpool,
            tc.tile_pool(name="work", bufs=2) as wpool,
            tc.tile_pool(name="ps", bufs=1, space="PSUM") as pspool,
        ):
            # ---- persistent tiles ----
            U1s = cpool.tile([128, T], F16, tag="u1")
            U2s = cpool.tile([128, T], F16, tag="u2")
            W8 = cpool.tile([128, 2], XTDT, tag="w8")
            Bs = cpool.tile([128, 2 * KT], F32, tag="bs")
            shift = cpool.tile([128, 1], F32, tag="shift")
            warm = cpool.tile([128, 128], F16, tag="warm")
            ones32 = cpool.tile([128, 1], F32, tag="ones32")
            ZP = cpool.tile([128, 8], F32, tag="zp")
            OST = cpool.tile([128, 32], F32, tag="ost")
            ZST = cpool.tile([32, 32], F32, tag="zst")
            OT = cpool.tile([32, 128], F32, tag="ot")
            OUT32 = cpool.tile([32, 128], F32, tag="out32")
            ZT = cpool.tile([32, 32], F32, tag="zt")
            C1p = cpool.tile([128, 2], F16, tag="c1p")
            C2p = cpool.tile([128, 2], F16, tag="c2p")
            C1s = cpool.tile([128, 2], F16, tag="c1s")  # cols: b2, b3
            C2s = cpool.tile([128, 2], F16, tag="c2s")

            nc.gpsimd.memset(shift[:], ET_SHIFT)
            nc.gpsimd.memset(warm[:], 0.0)
            nc.gpsimd.memset(ones32[:], 1.0)
            nc.gpsimd.memset(OST[:], 0.0)
            nc.gpsimd.memset(ZST[:], 0.0)
            # params via SWDGE: keeps the HWDGE rings free for x/U
            nc.gpsimd.dma_start(W8[:], w8d)
            nc.gpsimd.dma_start(Bs[:], bsd)

            # ---- PE warmup: release the HAM clock gate before work lands
            psW = pspool.tile([1, 128], F32, tag="psmall", bufs=1)
            for _ in range(N_WARM):
                nc.tensor.matmul(psW[:], warm[:, 0:1], warm[:], start=True, stop=True)

            # ---- x DMAs; U mid-stream; xT copies behind each batch ----
            X1 = [xpool.tile([128, KT, CW], F16, tag=f"x1_{b}", name=f"x1t{b}") for b in range(BPC)]
            X2 = [xpool.tile([128, KT, CW], F16, tag=f"x2_{b}", name=f"x2t{b}") for b in range(BPC)]
            XT1 = [xpool.tile([128, T], XTDT, tag=f"xt1_{b}", name=f"xt1t{b}") for b in range(BPC)]
            XT2 = [xpool.tile([128, T], XTDT, tag=f"xt2_{b}", name=f"xt2t{b}") for b in range(BPC)]

            KH = KT // 2
            def issue_t(xdt, Xt, b):
                xs = xdt[b].rearrange("(p k) c -> p k c", k=KT)
                nc.sync.dma_start(Xt[b][:, 0:KH, :], xs[:, 0:KH])
                nc.scalar.dma_start(Xt[b][:, KH:KT, :], xs[:, KH:KT])

            def issue_xt(b):
                nc.sync.dma_start(XT1[b][:], xt1d[b])
                nc.scalar.dma_start(XT2[b][:], xt2d[b])

            issue_t(x1d, X1, 0)
            issue_t(x2d, X2, 0)
            issue_t(x1d, X1, 1)
            issue_t(x2d, X2, 1)
            issue_xt(0)
            issue_xt(1)
            nc.sync.dma_start(U1s[:], u1d)
            nc.scalar.dma_start(U2s[:], u2d)
            issue_t(x1d, X1, 2)
            issue_t(x2d, X2, 2)
            issue_xt(2)
            issue_t(x1d, X1, 3)
            issue_t(x2d, X2, 3)
            issue_xt(3)

            psO = pspool.tile([128, 8], F32, tag="psO", bufs=1)

            EX = [None] * (2 * BPC)

            def gram_and_c(b):
                """PE Gram phases + c1/c2; G/t/c copies on ACT."""
                x1t, x2t = X1[b], X2[b]
                psA = pspool.tile([128, CW], F32, tag="psA", bufs=1)
                psB = pspool.tile([128, CW], F32, tag="psB", bufs=1)
                for k in range(KT):
                    nc.tensor.matmul(
                        psA[:], x1t[:, k, 0:E], x2t[:, k, :],
                        start=(k == 0), stop=(k == KT - 1),
                    )
                for k in range(KT):
                    nc.tensor.matmul(
                        psB[:], x2t[:, k, 0:E], x1t[:, k, :],
                        start=(k == 0), stop=(k == KT - 1),
                    )
                GA = wpool.tile([128, E], F16, tag="ga", bufs=2)
                GB = wpool.tile([128, E], F16, tag="gb", bufs=2)
                TC = wpool.tile([128, 2], F16, tag="tc", bufs=2)
                nc.scalar.copy(TC[:, 0:1], psA[:, E : E + 1])
                nc.scalar.copy(TC[:, 1:2], psB[:, E : E + 1])
                nc.scalar.copy(GA[:], psA[:, 0:E])
                nc.scalar.copy(GB[:], psB[:, 0:E])
                psC = pspool.tile([128, 4], F32, tag="psC", bufs=1)
                nc.tensor.matmul(psC[:, 0:2], GA[:], TC[:], start=True, stop=True)
                nc.tensor.matmul(psC[:, 2:4], GB[:], TC[:], start=True, stop=True)
                if b < 2:
                    d1, d2, col = C1p, C2p, b
                else:
                    d1, d2, col = C1s, C2s, b - 2
                nc.scalar.mul(d1[:, col : col + 1], psC[:, 1:2], INV_T)
                nc.scalar.mul(d2[:, col : col + 1], psC[:, 2:3], INV_T)

            def u_mm(bs_list, c1t, c2t):
                """PE et logits: per k-tile, U_k^T @ c accumulated with the
                per-batch xT8_k^T @ W8 column (x@W on the PE, start=False)."""
                n = len(bs_list)
                psE1 = pspool.tile([128, KT * n], F32, tag="psE1", bufs=1)
                psE2 = pspool.tile([128, KT * n], F32, tag="psE2", bufs=1)
                for s, (psE, Us, XTs, ct) in enumerate(
                    ((0, U1s, XT1, c1t), (1, U2s, XT2, c2t))
                ):
                    _, Us, XTs, ct = (s, Us, XTs, ct)
                    psE = psE1 if s == 0 else psE2
                    for k in range(KT):
                        ks = slice(k * 128, (k + 1) * 128)
                        nc.tensor.matmul(
                            psE[:, k * n : (k + 1) * n],
                            Us[:, ks], ct[:, 0:n],
                            start=True, stop=False, skip_group_check=True,
                        )
                        for j, b in enumerate(bs_list):
                            nc.tensor.matmul(
                                psE[:, k * n + j : k * n + j + 1],
                                XTs[b][:, ks], W8[:, s : s + 1],
                                start=False, stop=True, skip_group_check=True,
                            )
                return psE1, psE2

            def et_exp(psE1, psE2, n, j, b):
                """DVE et(+bias) + ACT exp for batch b (col j of the group)."""
                v1 = psE1.rearrange("p (k c) -> p k c", c=n)
                v2 = psE2.rearrange("p (k c) -> p k c", c=n)
                for s, vv in ((0, v1), (1, v2)):
                    et = wpool.tile([128, KT], F32, tag="et", bufs=4, name=f"et{b}{s}")
                    nc.vector.scalar_tensor_tensor(
                        out=et[:], in0=vv[:, :, j], scalar=1.0,
                        in1=Bs[:, s * KT : (s + 1) * KT], op0=ALU.mult, op1=ALU.add,
                    )
                    ex = wpool.tile([128, KT], BF16, tag=f"ex_{b}_{s}", bufs=1,
                                    name=f"ex{b}{s}")
                    jj = 2 * b + s
                    nc.scalar.activation(
                        ex[:], et[:], AF.Exp, bias=shift[:],
                        accum_out=ZP[:, jj : jj + 1],
                    )
                    EX[jj] = ex

            def readout(b):
                for s, xt in ((0, X1[b]), (1, X2[b])):
                    jj = 2 * b + s
                    exv = EX[jj]
                    for k in range(KT):
                        nc.tensor.matmul(
                            psO[:, jj : jj + 1],
                            xt[:, k, 0:E], exv[:, k : k + 1],
                            start=(k == 0), stop=(k == KT - 1),
                        )

            # ---- schedule ----
            gram_and_c(0)
            gram_and_c(1)
            pe1, pe2 = u_mm([0, 1], C1p, C2p)
            et_exp(pe1, pe2, 2, 0, 0)
            et_exp(pe1, pe2, 2, 1, 1)
            readout(0)
            readout(1)
            gram_and_c(2)
            pe3, pe4 = u_mm([2], C1s, C2s)
            et_exp(pe3, pe4, 1, 0, 2)
            readout(2)
            gram_and_c(3)
            pe5, pe6 = u_mm([3], C1s[:, 1:2], C2s[:, 1:2])
            et_exp(pe5, pe6, 1, 0, 3)
            # spin the PE warm through the wait for EX-b3 so the terminal
            # readout runs at 2.4GHz
            for _ in range(40):
                nc.tensor.matmul(psW[:], warm[:, 0:1], warm[:], start=True, stop=True)
            readout(3)

            # ---- finalize: Z chain, then transpose + 1/Z scale + store ----
            psZ = pspool.tile([1, 8], F32, tag="psmall", bufs=1)
            nc.tensor.matmul(psZ[:], ones32[:], ZP[:], start=True, stop=True)
            nc.vector.reciprocal(ZST[0:1, 0:8], psZ[:])
            nc.vector.transpose(ZT[:], ZST[:])
            nc.vector.tensor_copy(OST[:, 0:8], psO[:])
            for i in range(4):
                nc.vector.transpose(
                    OT[0:32, 32 * i : 32 * (i + 1)], OST[32 * i : 32 * (i + 1), 0:32]
                )
            nc.scalar.mul(OUT32[:], OT[:], ZT[:, 0:1])
            nc.sync.dma_start(outd, OUT32[0:8, :])

    return nc


_NC_CACHE = {}


def _get_nc():
    if "nc" not in _NC_CACHE:
        _NC_CACHE["nc"] = _build()
    return _NC_CACHE["nc"]


# U column permutation: tile k, lane j  <-  U[:, j*16 + k]
_UIDX = np.arange(T).reshape(128, KT).T.reshape(-1)


def _prep_in_maps(x1, x2, W1, b1, U1, W2, b2, U2):
    x1 = np.asarray(x1, dtype=np.float32)
    x2 = np.asarray(x2, dtype=np.float32)
    W1 = np.asarray(W1, dtype=np.float32)
    W2 = np.asarray(W2, dtype=np.float32)
    b1 = np.asarray(b1, dtype=np.float32)
    b2 = np.asarray(b2, dtype=np.float32)
    U1 = np.asarray(U1, dtype=np.float32)
    U2 = np.asarray(U2, dtype=np.float32)

    pad = np.zeros((B, T, 2), dtype=np.float32)
    pad[:, :, 0] = 1.0
    x1h = np.concatenate([x1, pad], axis=2).astype(np.float16)
    x2h = np.concatenate([x2, pad], axis=2).astype(np.float16)

    xtdt = _xt_np_dtype()
    # E-major copies, token columns permuted like U (tile k, lane j)
    x1t8 = np.ascontiguousarray(
        x1.transpose(0, 2, 1)[:, :, _UIDX].astype(xtdt)
    )
    x2t8 = np.ascontiguousarray(
        x2.transpose(0, 2, 1)[:, :, _UIDX].astype(xtdt)
    )
    w8 = np.ascontiguousarray(
        np.stack([W1[:, 0], W2[:, 0]], axis=1).astype(xtdt)
    )
    bs = np.ascontiguousarray(
        np.concatenate(
            [b1[:, 0].reshape(128, KT), b2[:, 0].reshape(128, KT)], axis=1
        )
    )
    u1p = np.ascontiguousarray(U1[:, _UIDX].astype(np.float16))
    u2p = np.ascontiguousarray(U2[:, _UIDX].astype(np.float16))

    in_maps = []
    for c in range(NCORES):
        sl = slice(c * BPC, (c + 1) * BPC)
        in_maps.append(
            {
                "x1c": np.ascontiguousarray(x1h[sl]),
                "x2c": np.ascontiguousarray(x2h[sl]),
                "x1t8": np.ascontiguousarray(x1t8[sl]),
                "x2t8": np.ascontiguousarray(x2t8[sl]),
                "u1": u1p,
                "u2": u2p,
                "w8": w8,
                "bs": bs,
            }
        )
    return in_maps


def _run(trace=False, tmpdir=None, **inputs):
    nc = _get_nc()
    if not _NC_CACHE.get("legalized"):
        _legalize_sync_waits(nc)
        _NC_CACHE["legalized"] = True
    in_maps = _prep_in_maps(**inputs)
    res = run_bass_kernel_spmd(
        nc, in_maps, list(range(NCORES)), trace=trace, tmpdir=tmpdir
    )
    out = np.concatenate(
        [r["out"].reshape(BPC, 2 * E) for r in res.results], axis=0
    )
    return out, res


def kernel(x1, x2, W1, b1, U1, W2, b2, U2):
    out, _ = _run(
        x1=x1, x2=x2, W1=W1, b1=b1, U1=U1, W2=W2, b2=b2, U2=U2
    )
    return out


# revision 20
# speedup vs baseline: 1.1601x; 1.1601x over previous
"""AttentionMM kernel for Trainium2 (Bass/Tile), data-parallel over 8 NeuronCores.

Math (per batch b, with x1,x2: (T,E)):
    S = x1 @ x2^T  is never materialized:
        psA = x1^T @ [x2|1] = [G2 | t1] ;  psB = x2^T @ [x1|1] = [G | t2]
        c1 = (1/T) G2^T t2 ;  c2 = (1/T) G t1
    et1 = c1 @ U1 + x1 @ W1 + b1 ;  et2 = c2 @ U2 + x2 @ W2 + b2
    o1 = softmax(et1) @ x1 ;  o2 = softmax(et2) @ x2 ;  out = [o1 | o2]

v3 (v2 was 54.4us; the f32r baseline 83-91us):
  - x/U/W fp16 (half DMA, 1-pass PE, hidden weight loads), EX bf16 with
    the constant-shift softmax (host numerics sim: ~3.0e-3 rel err).
  - x@W is the bottleneck resource (measured: DVE mult 1.23us, DVE
    reduce 2.27us per (T,E) tensor-batch; GPSIMD mult 4.1us, GPSIMD
    cannot reduce free axes). Split: GPSIMD owns the 4 x2 mults, DVE
    owns the 4 x1 mults + all 8 reduces, emitted in deadline order so
    the engine queues never idle ahead of a late dependency. The bias
    is added into one e-slice of the product pre-reduce (zeros in
    practice, kept for generality).
  - Column-form readout (1-col moving operand), Z via activation
    accum_out; batched finalize: DVE 32x32 block transposes + ACT
    1/Z scale + a single 4KB output store.
  - U-phase: pair (b0,b1) under the b2/b3 DMA shadow, then b2, b3 solo.
  - Params ride SWDGE; b0 lands in quarters so DVE/PE start ~1us
    earlier; U1/U2 mid-stream on the HWDGE rings.
  - 60 wide warmup matmuls release the PE HAM throttle before work
    lands; TileContext exit skips the (unneeded) SWDGE sem reset.
"""

import numpy as np

import concourse.bass as bass
import concourse.mybir as mybir
import concourse.tile as tile
from concourse.bass_utils import run_bass_kernel_spmd

B, T, E = 32, 2048, 128
NCORES = 8
BPC = B // NCORES
KT = T // 128                # token t = p*16 + k
CW = E + 2
F32 = mybir.dt.float32
F16 = mybir.dt.float16
BF16 = mybir.dt.bfloat16
AF = mybir.ActivationFunctionType
ALU = mybir.AluOpType
ET_SHIFT = -40.0
INV_T = 1.0 / T
N_WARM = 60


def _patch_sem_clear():
    """The installed walrus cannot encode EVENT_SEMAPHORE_RANGE_CLEAR (raw
    ISA, "ISA wrong length"), which TileContext's exit path emits via
    gpsimd.sem_clear. Skip the clear entirely (keep the bookkeeping): the
    runtime re-initializes semaphore state per NEFF execution, and the
    re-execution check in test.py guards this assumption."""
    if getattr(bass.Bass, "_semclear_patched", False):
        return

    def patched(self, sems):
        if not sems:
            return
        sem_nums = [s.num if hasattr(s, "num") else s for s in sems]
        self._state.prepend_free_semaphores(sem_nums)
        for poison_set in self._tile_sem_poison_stack:
            poison_set.update(sem_nums)

    bass.Bass.clear_and_free_semaphores = patched
    bass.Bass._semclear_patched = True


def _legalize_sync_waits(nc):
    """The installed walrus encodes at most one sync-wait per instruction
    ("Too many sync wait commands"). Move excess waits onto engine NoOps
    inserted immediately before the instruction — same engine, same
    program position, so semantics are unchanged."""
    import bass_rust

    fn = nc.m.functions[0]
    n_nops = 0
    for blk in fn.blocks:
        insts = blk.instructions
        out = []
        dirty = False
        for inst in insts:
            si = inst.sync_info
            if si is not None and len(si.on_wait) > 1:
                waits = list(si.on_wait)
                for w in waits[:-1]:
                    nop = mybir.InstNoOp(
                        name=f"waitnop-{n_nops}", engine=inst.engine
                    )
                    nop.sync_info = bass_rust.SyncInfo(
                        on_wait=[w], on_update=[]
                    )
                    out.append(nop)
                    n_nops += 1
                inst.sync_info = bass_rust.SyncInfo(
                    on_wait=[waits[-1]], on_update=list(si.on_update)
                )
                dirty = True
            out.append(inst)
        if dirty:
            blk.instructions = out
    return n_nops


def _build():
    _patch_sem_clear()
    nc = bass.Bass(
        "TRN2", target_bir_lowering=False, debug=False, num_devices=NCORES
    )

    x1d = nc.dram_tensor("x1c", (BPC, T, CW), F16, kind="ExternalInput").ap()
    x2d = nc.dram_tensor("x2c", (BPC, T, CW), F16, kind="ExternalInput").ap()
    u1d = nc.dram_tensor("u1", (E, T), F16, kind="ExternalInput").ap()
    u2d = nc.dram_tensor("u2", (E, T), F16, kind="ExternalInput").ap()
    wbcd = nc.dram_tensor("wbc", (128, 2 * E), F16, kind="ExternalInput").ap()
    bsd = nc.dram_tensor("bs", (128, 2 * KT), F32, kind="ExternalInput").ap()
    outd = nc.dram_tensor("out", (2 * BPC, E), F32, kind="ExternalOutput").ap()

    with tile.TileContext(nc) as tc:
        with (
            tc.tile_pool(name="const", bufs=1) as cpool,
            tc.tile_pool(name="xpool", bufs=1) as xpool,
            tc.tile_pool(name="work", bufs=2) as wpool,
            tc.tile_pool(name="ps", bufs=1, space="PSUM") as pspool,
        ):
            # ---- persistent tiles ----
            U1s = cpool.tile([128, T], F16, tag="u1")
            U2s = cpool.tile([128, T], F16, tag="u2")
            Wbc = cpool.tile([128, 2 * E], F16, tag="wbc")
            Bs = cpool.tile([128, 2 * KT], F32, tag="bs")
            shift = cpool.tile([128, 1], F32, tag="shift")
            warm = cpool.tile([128, 128], F16, tag="warm")
            ones32 = cpool.tile([128, 1], F32, tag="ones32")
            ZP = cpool.tile([128, 8], F32, tag="zp")
            OST = cpool.tile([128, 32], F32, tag="ost")
            ZST = cpool.tile([32, 32], F32, tag="zst")
            OT = cpool.tile([32, 128], F32, tag="ot")
            OUT32 = cpool.tile([32, 128], F32, tag="out32")
            ZT = cpool.tile([32, 32], F32, tag="zt")
            C1p = cpool.tile([128, 2], F16, tag="c1p")
            C2p = cpool.tile([128, 2], F16, tag="c2p")
            C1s = cpool.tile([128, 2], F16, tag="c1s")  # cols: b2, b3
            C2s = cpool.tile([128, 2], F16, tag="c2s")

            nc.gpsimd.memset(shift[:], ET_SHIFT)
            nc.gpsimd.memset(warm[:], 0.0)
            nc.gpsimd.memset(ones32[:], 1.0)
            nc.gpsimd.memset(OST[:], 0.0)
            nc.gpsimd.memset(ZST[:], 0.0)
            # params via SWDGE: keeps the HWDGE rings free for x/U
            nc.gpsimd.dma_start(Wbc[:], wbcd)
            nc.gpsimd.dma_start(Bs[:], bsd)

            # ---- PE warmup: release the HAM clock gate before work lands
            psW = pspool.tile([1, 128], F32, tag="psmall", bufs=1)
            for _ in range(N_WARM):
                nc.tensor.matmul(psW[:], warm[:, 0:1], warm[:], start=True, stop=True)

            # ---- x DMAs; U mid-stream ----
            X1 = [xpool.tile([128, KT, CW], F16, tag=f"x1_{b}", name=f"x1t{b}") for b in range(BPC)]
            X2 = [xpool.tile([128, KT, CW], F16, tag=f"x2_{b}", name=f"x2t{b}") for b in range(BPC)]

            # each tensor's two halves ride BOTH rings concurrently, x1
            # ahead of x2: the DVE (x1) chain starts earlier and mid-chain
            # landings outpace the in-order DVE queue
            KH = KT // 2
            def issue_t(xdt, Xt, b):
                xs = xdt[b].rearrange("(p k) c -> p k c", k=KT)
                nc.sync.dma_start(Xt[b][:, 0:KH, :], xs[:, 0:KH])
                nc.scalar.dma_start(Xt[b][:, KH:KT, :], xs[:, KH:KT])

            issue_t(x1d, X1, 0)
            issue_t(x2d, X2, 0)
            issue_t(x1d, X1, 1)
            issue_t(x2d, X2, 1)
            nc.sync.dma_start(U1s[:], u1d)
            nc.scalar.dma_start(U2s[:], u2d)
            issue_t(x1d, X1, 2)
            issue_t(x2d, X2, 2)
            issue_t(x1d, X1, 3)
            issue_t(x2d, X2, 3)

            psO = pspool.tile([128, 8], F32, tag="psO", bufs=1)

            EX = [None] * (2 * BPC)
            XWB = [None] * (2 * BPC)
            SCR = [None] * (2 * BPC)

            def gram_and_c(b):
                """PE Gram phases + c1/c2; G/t/c copies on ACT."""
                x1t, x2t = X1[b], X2[b]
                psA = pspool.tile([128, CW], F32, tag="psA", bufs=1)
                psB = pspool.tile([128, CW], F32, tag="psB", bufs=1)
                for k in range(KT):
                    nc.tensor.matmul(
                        psA[:], x1t[:, k, 0:E], x2t[:, k, :],
                        start=(k == 0), stop=(k == KT - 1),
                    )
                for k in range(KT):
                    nc.tensor.matmul(
                        psB[:], x2t[:, k, 0:E], x1t[:, k, :],
                        start=(k == 0), stop=(k == KT - 1),
                    )
                GA = wpool.tile([128, E], F16, tag="ga", bufs=2)
                GB = wpool.tile([128, E], F16, tag="gb", bufs=2)
                TC = wpool.tile([128, 2], F16, tag="tc", bufs=2)
                nc.scalar.copy(TC[:, 0:1], psA[:, E : E + 1])
                nc.scalar.copy(TC[:, 1:2], psB[:, E : E + 1])
                nc.scalar.copy(GA[:], psA[:, 0:E])
                nc.scalar.copy(GB[:], psB[:, 0:E])
                psC = pspool.tile([128, 4], F32, tag="psC", bufs=1)
                nc.tensor.matmul(psC[:, 0:2], GA[:], TC[:], start=True, stop=True)
                nc.tensor.matmul(psC[:, 2:4], GB[:], TC[:], start=True, stop=True)
                if b < 2:
                    d1, d2, col = C1p, C2p, b
                else:
                    d1, d2, col = C1s, C2s, b - 2
                # 1/T scale + fp16 cast on ACT
                nc.scalar.mul(d1[:, col : col + 1], psC[:, 1:2], INV_T)
                nc.scalar.mul(d2[:, col : col + 1], psC[:, 2:3], INV_T)

            def xw_mult(b, s, chunks=1):
                """x_s @ W_s product for batch b: s=0 on DVE, s=1 on GPSIMD.
                chunks>1 splits along k so work starts on partial DMAs."""
                xt = (X1 if s == 0 else X2)[b]
                eng = nc.vector if s == 0 else nc.gpsimd
                scr = wpool.tile([128, KT, E], F16, tag=f"scr{s}", bufs=2,
                                 name=f"scr{s}_{b}")
                kq = KT // chunks
                for h in range(chunks):
                    ks = slice(h * kq, (h + 1) * kq)
                    wv = Wbc[:, s * E : (s + 1) * E].unsqueeze(1).broadcast_to(
                        (128, kq, E)
                    )
                    eng.tensor_tensor(scr[:, ks, :], xt[:, ks, 0:E], wv, ALU.mult)
                SCR[2 * b + s] = scr

            def xw_reduce(b, s, chunks=1):
                xwb = wpool.tile([128, KT], F32, tag=f"xwb_{b}_{s}", bufs=1,
                                 name=f"xwb{b}{s}")
                kq = KT // chunks
                for h in range(chunks):
                    ks = slice(h * kq, (h + 1) * kq)
                    nc.vector.tensor_reduce(
                        out=xwb[:, ks], in_=SCR[2 * b + s][:, ks, :],
                        axis=mybir.AxisListType.X, op=ALU.add,
                    )
                XWB[2 * b + s] = xwb

            def gp_fold(b):
                """GPSIMD fold-tree of scr2_b down to 16 e-columns; the cheap
                tail reduce stays on DVE (emitted separately)."""
                scr = SCR[2 * b + 1]
                fA = wpool.tile([128, KT, 64], F16, tag="gfA", bufs=2, name=f"gfA{b}")
                fB = wpool.tile([128, KT, 32], F16, tag="gfB", bufs=2, name=f"gfB{b}")
                fC = wpool.tile([128, KT, 16], F16, tag="gfC", bufs=2, name=f"gfC{b}")
                nc.gpsimd.tensor_tensor(fA[:], scr[:, :, 0:64], scr[:, :, 64:128], ALU.add)
                nc.gpsimd.tensor_tensor(fB[:], fA[:, :, 0:32], fA[:, :, 32:64], ALU.add)
                nc.gpsimd.tensor_tensor(fC[:], fB[:, :, 0:16], fB[:, :, 16:32], ALU.add)
                SCR[2 * b + 1] = fC

            def tail_reduce(b):
                xwb = wpool.tile([128, KT], F32, tag=f"xwb_{b}_1", bufs=1,
                                 name=f"xwbt{b}1")
                nc.vector.tensor_reduce(
                    out=xwb[:], in_=SCR[2 * b + 1][:], axis=mybir.AxisListType.X,
                    op=ALU.add,
                )
                XWB[2 * b + 1] = xwb

            def u_mm(bs_list, c1t, c2t):
                """PE et logits for the batches in bs_list."""
                n = len(bs_list)
                psE1 = pspool.tile([128, KT * n], F32, tag="psE1", bufs=1)
                psE2 = pspool.tile([128, KT * n], F32, tag="psE2", bufs=1)
                for k in range(KT):
                    nc.tensor.matmul(
                        psE1[:, k * n : (k + 1) * n],
                        U1s[:, k * 128 : (k + 1) * 128],
                        c1t[:, 0:n], start=True, stop=True,
                    )
                for k in range(KT):
                    nc.tensor.matmul(
                        psE2[:, k * n : (k + 1) * n],
                        U2s[:, k * 128 : (k + 1) * 128],
                        c2t[:, 0:n], start=True, stop=True,
                    )
                return psE1, psE2

            def et_exp(psE1, psE2, n, j, b):
                """DVE et assembly + ACT exp for batch b (col j of the pair)."""
                v1 = psE1.rearrange("p (k c) -> p k c", c=n)
                v2 = psE2.rearrange("p (k c) -> p k c", c=n)
                for s, vv in ((0, v1), (1, v2)):
                    et0 = wpool.tile([128, KT], F32, tag="et0", bufs=4, name=f"et0{b}{s}")
                    et = wpool.tile([128, KT], F32, tag="et", bufs=4, name=f"et{b}{s}")
                    nc.vector.scalar_tensor_tensor(
                        out=et0[:], in0=vv[:, :, j], scalar=1.0,
                        in1=XWB[2 * b + s][:], op0=ALU.mult, op1=ALU.add,
                    )
                    nc.vector.scalar_tensor_tensor(
                        out=et[:], in0=et0[:], scalar=1.0,
                        in1=Bs[:, s * KT : (s + 1) * KT], op0=ALU.mult, op1=ALU.add,
                    )
                    ex = wpool.tile([128, KT], BF16, tag=f"ex_{b}_{s}", bufs=1,
                                    name=f"ex{b}{s}")
                    jj = 2 * b + s
                    nc.scalar.activation(
                        ex[:], et[:], AF.Exp, bias=shift[:],
                        accum_out=ZP[:, jj : jj + 1],
                    )
                    EX[jj] = ex

            def et_exp1(psE, n, j, b, s):
                vv = psE.rearrange("p (k c) -> p k c", c=n)
                et0 = wpool.tile([128, KT], F32, tag="et0", bufs=4,
                                 name=f"et0{b}{s}")
                et = wpool.tile([128, KT], F32, tag="et", bufs=4,
                                name=f"et{b}{s}")
                nc.vector.scalar_tensor_tensor(
                    out=et0[:], in0=vv[:, :, j], scalar=1.0,
                    in1=XWB[2 * b + s][:], op0=ALU.mult, op1=ALU.add,
                )
                nc.vector.scalar_tensor_tensor(
                    out=et[:], in0=et0[:], scalar=1.0,
                    in1=Bs[:, s * KT : (s + 1) * KT], op0=ALU.mult, op1=ALU.add,
                )
                ex = wpool.tile([128, KT], BF16, tag=f"ex_{b}_{s}", bufs=1,
                                name=f"ex{b}{s}")
                jj = 2 * b + s
                nc.scalar.activation(
                    ex[:], et[:], AF.Exp, bias=shift[:],
                    accum_out=ZP[:, jj : jj + 1],
                )
                EX[jj] = ex

            def readout(b):
                for s, xt in ((0, X1[b]), (1, X2[b])):
                    jj = 2 * b + s
                    exv = EX[jj]
                    for k in range(KT):
                        nc.tensor.matmul(
                            psO[:, jj : jj + 1],
                            xt[:, k, 0:E], exv[:, k : k + 1],
                            start=(k == 0), stop=(k == KT - 1),
                        )

            # ---- schedule (per-engine queues run in emission order) ----
            xw_mult(0, 1)            # gpsimd: m2b0
            xw_mult(1, 1)            # gpsimd: m2b1
            xw_mult(2, 1)            # gpsimd: m2b2
            xw_mult(3, 1)            # gpsimd: m2b3

            xw_mult(0, 0)            # dve: m1b0
            xw_reduce(0, 0)
            gram_and_c(0)            # pe + act
            xw_mult(1, 0)            # dve: m1b1
            xw_reduce(0, 1)          # dve: r2b0
            gram_and_c(1)
            xw_reduce(1, 0)          # dve: r1b1
            xw_reduce(1, 1)          # dve: r2b1
            pe1, pe2 = u_mm([0, 1], C1p, C2p)
            xw_mult(2, 0)            # dve: m1b2
            gram_and_c(2)
            xw_reduce(2, 0)
            et_exp(pe1, pe2, 2, 0, 0)
            et_exp(pe1, pe2, 2, 1, 1)
            readout(0)
            readout(1)
            pe3, pe4 = u_mm([2], C1s, C2s)
            xw_mult(3, 0)            # dve: m1b3
            gram_and_c(3)
            xw_reduce(3, 0)
            xw_reduce(2, 1)
            pe5, pe6 = u_mm([3], C1s[:, 1:2], C2s[:, 1:2])
            et_exp(pe3, pe4, 1, 0, 2)
            readout(2)
            # b3 s=0 needs only r1b3: its exp/readout overlap r2b3
            et_exp1(pe5, 1, 0, 3, 0)
            xw_reduce(3, 1)
            et_exp1(pe6, 1, 0, 3, 1)
            # spin the PE warm through the wait for EX-b3 so the terminal
            # readout runs at 2.4GHz (8 early warms re-throttled by then)
            for _ in range(60):
                nc.tensor.matmul(psW[:], warm[:, 0:1], warm[:], start=True, stop=True)
            readout(3)

            # ---- finalize: Z chain overlaps the terminal readout ----
            psZ = pspool.tile([1, 8], F32, tag="psmall", bufs=1)
            nc.tensor.matmul(psZ[:], ones32[:], ZP[:], start=True, stop=True)
            nc.vector.reciprocal(ZST[0:1, 0:8], psZ[:])
            nc.vector.transpose(ZT[:], ZST[:])
            nc.vector.tensor_copy(OST[:, 0:8], psO[:])
            for i in range(4):
                nc.vector.transpose(
                    OT[0:32, 32 * i : 32 * (i + 1)], OST[32 * i : 32 * (i + 1), 0:32]
                )
            nc.scalar.mul(OUT32[:], OT[:], ZT[:, 0:1])
            nc.sync.dma_start(outd, OUT32[0:8, :])

    return nc


_NC_CACHE = {}


def _get_nc():
    if "nc" not in _NC_CACHE:
        _NC_CACHE["nc"] = _build()
    return _NC_CACHE["nc"]


# U column permutation: tile k, lane j  <-  U[:, j*16 + k]
_UIDX = np.arange(T).reshape(128, KT).T.reshape(-1)


def _prep_in_maps(x1, x2, W1, b1, U1, W2, b2, U2):
    x1 = np.asarray(x1, dtype=np.float32)
    x2 = np.asarray(x2, dtype=np.float32)
    W1 = np.asarray(W1, dtype=np.float32)
    W2 = np.asarray(W2, dtype=np.float32)
    b1 = np.asarray(b1, dtype=np.float32)
    b2 = np.asarray(b2, dtype=np.float32)
    U1 = np.asarray(U1, dtype=np.float32)
    U2 = np.asarray(U2, dtype=np.float32)

    pad = np.zeros((B, T, 2), dtype=np.float32)
    pad[:, :, 0] = 1.0
    x1h = np.concatenate([x1, pad], axis=2).astype(np.float16)
    x2h = np.concatenate([x2, pad], axis=2).astype(np.float16)

    wbc = np.ascontiguousarray(
        np.broadcast_to(
            np.concatenate([W1[:, 0], W2[:, 0]])[None, :], (128, 2 * E)
        ).astype(np.float16)
    )
    bs = np.ascontiguousarray(
        np.concatenate(
            [b1[:, 0].reshape(128, KT), b2[:, 0].reshape(128, KT)], axis=1
        )
    )
    u1p = np.ascontiguousarray(U1[:, _UIDX].astype(np.float16))
    u2p = np.ascontiguousarray(U2[:, _UIDX].astype(np.float16))

    in_maps = []
    for c in range(NCORES):
        sl = slice(c * BPC, (c + 1) * BPC)
        in_maps.append(
            {
                "x1c": np.ascontiguousarray(x1h[sl]),
                "x2c": np.ascontiguousarray(x2h[sl]),
                "u1": u1p,
                "u2": u2p,
                "wbc": wbc,
                "bs": bs,
            }
        )
    return in_maps


def _run(trace=False, tmpdir=None, **inputs):
    nc = _get_nc()
    if not _NC_CACHE.get("legalized"):
        _legalize_sync_waits(nc)
        _NC_CACHE["legalized"] = True
    in_maps = _prep_in_maps(**inputs)
    res = run_bass_kernel_spmd(
        nc, in_maps, list(range(NCORES)), trace=trace, tmpdir=tmpdir
    )
    out = np.concatenate(
        [r["out"].reshape(BPC, 2 * E) for r in res.results], axis=0
    )
    return out, res


def kernel(x1, x2, W1, b1, U1, W2, b2, U2):
    out, _ = _run(
        x1=x1, x2=x2, W1=W1, b1=b1, U1=U1, W2=W2, b2=b2, U2=U2
    )
    return out
